# revision 26
# baseline (speedup 1.0000x reference)
"""Trainium2 Bass kernel for a dense transformer block.

Shapes (hardcoded): x [16, 1024, 768], 12 heads, head_dim 64, MLP hidden 3072.
Sharding: data-parallel over batch, 2 batches (2048 tokens) per core x 8 cores.

Per-core program layout strategy:
  - activations transposed [feature, token] for matmuls (contraction on
    partitions), natural [token, feature] for LayerNorm stats/apply
  - LN affine (g, b) and attention scale folded into weights host-side
  - S computed transposed: S^T[k, q] = k_h^T.T @ q_h^T; exp on ScalarE
    (logits bounded, no max subtraction needed); ctx natural via
    lhsT = P^T chunk, rhs = [v | 1] so the softmax denominator falls out of
    the same matmul's extra column; normalization is a per-partition scalar.
  - weights bf16, PSUM accumulation f32, residual path f32.
"""

import math

import numpy as np
import ml_dtypes

import concourse.bass as bass
import concourse.mybir as mybir
import concourse.tile as tile
from concourse import bacc
from concourse.bass_utils import run_bass_kernel_spmd

P = 128
C = 768
CC = C // P            # 6 feature chunks
T = 2048               # tokens per core (2 batches x 1024)
TT = T // P            # 16 token tiles
SEQ = 1024
NB = 2                 # batches per core
KC = SEQ // P          # 8 key chunks per batch
H = 12
D = 64
HID = 3072
HC = HID // P          # 24 hidden chunks
EPS = 1e-5
SCALE = D ** -0.5
N_CORES = 8

F32 = mybir.dt.float32
BF16 = mybir.dt.bfloat16

AF = mybir.ActivationFunctionType
ALU = mybir.AluOpType


def _legalize_sync_commands(nc):
    """Walrus caps every instruction (and DMA descriptor) at 2 total sync
    commands (waits + updates). Tile's wait pass can emit more when an
    instruction's required vector clock spans several procs. Relay pattern:
    move each excess wait onto a same-engine NoOp that bumps a per-engine
    relay semaphore; the instruction then waits only on the relay. For engine
    instructions this is semantically identical (the sequencer stalls at the
    same stream position either way); for DMA descriptors it shifts the wait
    from the queue to the issuing engine."""
    import bass_rust

    used = set()
    for i in nc.inst_map.values():
        si = getattr(i, "sync_info", None)
        if si is None:
            continue
        for s in si.on_wait or []:
            used.add(s.id)
        for u in si.on_update or []:
            used.add(u.id)
    relay = [max(used) + 1, 0]  # [sem_id, count]
    assert relay[0] < 250, "out of relay semaphores"
    skip = (mybir.InstDrain, mybir.InstNoOp, mybir.InstEventSemaphore)
    n_fixed = 0
    for f in nc.m.functions:
        for b in f.blocks:
            insts = b.instructions
            pos = 0
            while pos < len(insts):
                ins = insts[pos]
                si = getattr(ins, "sync_info", None)
                if (
                    si is not None
                    and not isinstance(ins, skip)
                    and getattr(ins, "engine", None) is not None
                    and len(si.on_wait or []) + len(si.on_update or []) > 2
                ):
                    n_fixed += 1
                    for w in si.on_wait:
                        assert w.wait_reg is None, "reg-mode wait not relayable"
                        relay[1] += 1
                        nop = mybir.InstNoOp(
                            name=f"sync-relay-{relay[1]}",
                            engine=mybir.EngineType.SP,
                            ins=[],
                            outs=[],
                            sync_info=bass_rust.SyncInfo(
                                on_wait=[w],
                                on_update=[
                                    bass_rust.SyncUpdate(
                                        sync_type="semaphore",
                                        id=relay[0],
                                        ant_name=f"relay_{relay[0]}",
                                        update_mode="sem-add-imm",
                                        update_value=1,
                                        update_reg=None,
                                    )
                                ],
                            ),
                        )
                        insts.insert(pos, nop)
                        pos += 1
                    ins.sync_info = bass_rust.SyncInfo(
                        on_wait=[
                            bass_rust.SyncWait(
                                sync_type="semaphore",
                                id=relay[0],
                                ant_name=f"relay_{relay[0]}",
                                wait_mode="sem-ge-imm",
                                wait_value=relay[1],
                                wait_reg=None,
                            )
                        ],
                        on_update=list(si.on_update or []),
                    )
                pos += 1
    return n_fixed


def build_nc(trace_scopes=False):
    nc = bacc.Bacc("TRN2", target_bir_lowering=False)

    x_d = nc.dram_tensor("x", [T, C], F32, kind="ExternalInput")
    wq_d = nc.dram_tensor("wq", [P, CC, C], BF16, kind="ExternalInput")
    wk_d = nc.dram_tensor("wk", [P, CC, C], BF16, kind="ExternalInput")
    wv_d = nc.dram_tensor("wv", [P, CC, C], BF16, kind="ExternalInput")
    wo_d = nc.dram_tensor("wo", [P, CC, C], BF16, kind="ExternalInput")
    w1_d = nc.dram_tensor("w1", [P, CC, HID], BF16, kind="ExternalInput")
    w2_d = nc.dram_tensor("w2", [P, HC, C], BF16, kind="ExternalInput")
    bq_d = nc.dram_tensor("bq", [P, CC], F32, kind="ExternalInput")
    bk_d = nc.dram_tensor("bk", [P, CC], F32, kind="ExternalInput")
    b1_d = nc.dram_tensor("b1", [P, HC], F32, kind="ExternalInput")
    bv_d = nc.dram_tensor("bv", [1, C], BF16, kind="ExternalInput")
    bo_d = nc.dram_tensor("bo", [1, C], BF16, kind="ExternalInput")
    b2_d = nc.dram_tensor("b2", [1, C], BF16, kind="ExternalInput")
    ones_d = nc.dram_tensor("ones1", [1, P], BF16, kind="ExternalInput")
    ident_d = nc.dram_tensor("ident", [P, P], BF16, kind="ExternalInput")
    out_d = nc.dram_tensor("out", [T, C], F32, kind="ExternalOutput")

    TB = TT // NB  # 8 token tiles per batch

    with tile.TileContext(nc) as tc:
        with (
            tc.tile_pool(name="wts", bufs=1) as wts,
            tc.tile_pool(name="big", bufs=1) as big,
            tc.tile_pool(name="work", bufs=2) as work,
            tc.tile_pool(name="small", bufs=4) as small,
            tc.tile_pool(name="psS", bufs=2, space="PSUM") as psS,
            tc.tile_pool(name="pden", bufs=2, space="PSUM") as pden,
            tc.tile_pool(name="pmix", bufs=2, space="PSUM") as pmix,
            tc.tile_pool(name="dram", bufs=1, space="DRAM") as dpool,
        ):
            # ---- constants / early weights ----
            ident = wts.tile([P, P], BF16, tag="ident", name="ident")
            nc.sync.dma_start(ident, ident_d[:, :])
            ones1 = wts.tile([1, P], BF16, tag="ones1", name="ones1")
            nc.sync.dma_start(ones1, ones_d[:, :])
            eps_t = wts.tile([P, 1], F32, tag="eps", name="eps_t")
            nc.vector.memset(eps_t, EPS)

            def load_split(tile_ap, dram_ap, n):
                d = tile_ap.shape[1]
                step = d // n
                for j in range(n):
                    nc.sync.dma_start(
                        tile_ap[:, j * step:(j + 1) * step],
                        dram_ap[:, j * step:(j + 1) * step],
                    )

            def load_x_tile(tt, src=x_d, tag="xt"):
                xt = work.tile([P, C], F32, tag=tag, name="xt", bufs=3)
                for j in range(4):
                    nc.sync.dma_start(
                        xt[:, j * 192:(j + 1) * 192],
                        src[tt * P:(tt + 1) * P, j * 192:(j + 1) * 192],
                    )
                return xt

            pre_x = {t: None for t in range(2)}
            for t in range(2):
                pre_x[t] = load_x_tile(t)

            wq = wts.tile([P, CC, C], BF16, tag="wq", name="wq",
                          padded_shape=[P, CC, HC * P // CC * 2])
            load_split(wq, wq_d, CC)
            wk = wts.tile([P, CC, C], BF16, tag="wk", name="wk")
            load_split(wk, wk_d, CC)
            wv = wts.tile([P, CC, C], BF16, tag="wv", name="wv")
            load_split(wv, wv_d, CC)
            bq = wts.tile([P, CC], F32, tag="bq", name="bq")
            nc.sync.dma_start(bq, bq_d[:, :])
            bk = wts.tile([P, CC], F32, tag="bk", name="bk")
            nc.sync.dma_start(bk, bk_d[:, :])
            bvr = wts.tile([1, C], BF16, tag="bvr", name="bvr")
            nc.sync.dma_start(bvr, bv_d[:, :])
            bor = wts.tile([1, C], BF16, tag="bor", name="bor")
            nc.sync.dma_start(bor, bo_d[:, :])
            b2r = wts.tile([1, C], BF16, tag="b2r", name="b2r")
            nc.sync.dma_start(b2r, b2_d[:, :])
            b1c = wts.tile([P, HC], F32, tag="b1c", name="b1c")
            nc.sync.dma_start(b1c, b1_d[:, :])

            # ---- DRAM scratch for batch-1 qkv staging + x2 spill ----
            qT_s = dpool.tile([C, SEQ], BF16, name="qT_s")
            kT_s = dpool.tile([C, SEQ], BF16, name="kT_s")
            v_s = dpool.tile([SEQ, H, D + 1], BF16, name="v_s")
            x2_s = dpool.tile([T, C], F32, name="x2_s")

            def layernorm_to(x_tile, hn_tile):
                st = small.tile([P, 3, 6], F32, tag="bnst", name="st")
                for sg in range(3):
                    nc.vector.bn_stats(st[:, sg], x_tile[:, sg * 256:(sg + 1) * 256])
                mv = small.tile([P, 2], F32, tag="mv", name="mv")
                nc.vector.bn_aggr(mv, st)
                rstd = small.tile([P, 1], F32, tag="rstd", name="rstd")
                nc.scalar.activation(rstd, mv[:, 1:2], AF.Sqrt, bias=eps_t, scale=1.0)
                nc.vector.reciprocal(rstd, rstd)
                nc.vector.tensor_scalar(
                    hn_tile, x_tile, mv[:, 0:1], rstd, op0=ALU.subtract, op1=ALU.mult
                )

            def transpose_batch(dst_ap, srcs, pool=None, tag="pden"):
                pool = pool if pool is not None else pden
                pt = pool.tile([P, 8, P], BF16, tag=tag, name="pt")
                for j, s in enumerate(srcs):
                    nc.tensor.transpose(pt[:, j], s, ident)
                nc.vector.tensor_copy(dst_ap, pt[:, : len(srcs)])

            def ln1_tile(t_global, hT, t8, pre=None):
                xt = pre if pre is not None else load_x_tile(t_global)
                hn = work.tile([P, C], BF16, tag="hn", name="hn", bufs=1)
                layernorm_to(xt, hn)
                transpose_batch(
                    hT[:, :, t8 * P:(t8 + 1) * P],
                    [hn[:, c * P:(c + 1) * P] for c in range(CC)],
                )

            def v_matmuls(hT, t8):
                """returns the two psum halves for v of token tile t8."""
                pmv = [
                    pden.tile([P, 384], F32, tag="pden", name="pmv")
                    for _ in range(2)
                ]
                for ci in range(CC):
                    for ha in range(2):
                        nc.tensor.matmul(
                            pmv[ha],
                            lhsT=hT[:, ci, t8 * P:(t8 + 1) * P],
                            rhs=wv[:, ci, ha * 384:(ha + 1) * 384],
                            start=(ci == 0),
                            stop=False,
                        )
                for ha in range(2):
                    nc.tensor.matmul(
                        pmv[ha],
                        lhsT=ones1,
                        rhs=bvr[:, ha * 384:(ha + 1) * 384],
                        start=False,
                        stop=True,
                    )
                return pmv

            def qk_matmul(wsb, hT, co, tb):
                pm = pden.tile([P, 512], F32, tag="pden", name="pm")
                for ci in range(CC):
                    nc.tensor.matmul(
                        pm,
                        lhsT=wsb[:, ci, co * P:(co + 1) * P],
                        rhs=hT[:, ci, tb * 512:(tb + 1) * 512],
                        start=(ci == 0),
                        stop=(ci == CC - 1),
                    )
                return pm

            def attention(qT, kT, vb, ctxT):
                for hp in range(CC):
                    stage = work.tile(
                        [P, KC, 2, D], BF16, tag="cstage", name="stage", bufs=1
                    )
                    for hi in range(2):
                        h = hp * 2 + hi
                        po = (h % 2) * D
                        hc2 = h // 2
                        ct = [
                            pmix.tile([P, 4, D + 1], F32, tag="pmix", name="ct")
                            for _ in range(2)
                        ]
                        for kc in range(KC):
                            pk = work.tile([P, SEQ], BF16, tag="pk", name="pk")
                            ps = psS.tile([P, SEQ], F32, tag="psS", name="ps")
                            for q2 in range(2):
                                nc.tensor.matmul(
                                    ps[:, q2 * 512:(q2 + 1) * 512],
                                    lhsT=kT[po:po + D, hc2, kc * P:(kc + 1) * P],
                                    rhs=qT[po:po + D, hc2, q2 * 512:(q2 + 1) * 512],
                                    start=True,
                                    stop=True,
                                )
                            nc.scalar.activation(pk, ps, AF.Exp)
                            for qt in range(KC):
                                nc.tensor.matmul(
                                    ct[qt // 4][:, qt % 4],
                                    lhsT=pk[:, qt * P:(qt + 1) * P],
                                    rhs=vb[:, kc, h, :],
                                    start=(kc == 0),
                                    stop=(kc == KC - 1),
                                )
                        for qt in range(KC):
                            rc = small.tile([P, 1], F32, tag="rc", name="rc")
                            nc.vector.reciprocal(rc, ct[qt // 4][:, qt % 4, D:])
                            nc.vector.tensor_scalar_mul(
                                stage[:, qt, hi], ct[qt // 4][:, qt % 4, :D], rc
                            )
                    transpose_batch(
                        ctxT[:, hp, :].rearrange("p (a b) -> p a b", b=P),
                        [stage[:, qt] for qt in range(KC)],
                        pool=pmix,
                        tag="pmix",
                    )

            def proj_ln2(ctxT, wo, b, x2buf_h2T):
                x2ts = []
                h2T = x2buf_h2T
                for t8 in range(TB):
                    xt = load_x_tile(b * TB + t8)
                    # alternate psum pools per tile so slot recycling (gated by
                    # the DVE residual-add + LN2 chain) never stalls PE
                    pool, ptag = (pden, "pden") if t8 % 2 == 0 else (psS, "psS")
                    pmo = [
                        pool.tile([P, 384], F32, tag=ptag, name="pmo")
                        for _ in range(2)
                    ]
                    for ci in range(CC):
                        for ha in range(2):
                            nc.tensor.matmul(
                                pmo[ha],
                                lhsT=ctxT[:, ci, t8 * P:(t8 + 1) * P],
                                rhs=wo[:, ci, ha * 384:(ha + 1) * 384],
                                start=(ci == 0),
                                stop=False,
                            )
                    x2t = work.tile([P, C], F32, tag="x2t", name="x2t")
                    for ha in range(2):
                        nc.tensor.matmul(
                            pmo[ha],
                            lhsT=ones1,
                            rhs=bor[:, ha * 384:(ha + 1) * 384],
                            start=False,
                            stop=True,
                        )
                        nc.vector.tensor_tensor(
                            x2t[:, ha * 384:(ha + 1) * 384],
                            pmo[ha],
                            xt[:, ha * 384:(ha + 1) * 384],
                            ALU.add,
                        )
                    for j in range(2):
                        nc.sync.dma_start(
                            x2_s[(b * TB + t8) * P:(b * TB + t8 + 1) * P,
                                 j * 384:(j + 1) * 384],
                            x2t[:, j * 384:(j + 1) * 384],
                        )
                    h2n = work.tile([P, C], BF16, tag="hn", name="h2n", bufs=1)
                    layernorm_to(x2t, h2n)
                    transpose_batch(
                        h2T[:, :, t8 * P:(t8 + 1) * P],
                        [h2n[:, c * P:(c + 1) * P] for c in range(CC)],
                    )
                return h2T

            def mlp(h2T, w1, w2, b):
                for tb4 in range(4):
                    gt = wts.tile([P, HC, 256], BF16, tag="wq", name="gt")
                    for hc in range(HC):
                        pm1 = pden.tile([P, 256], F32, tag="pden", name="pm1")
                        for ci in range(CC):
                            nc.tensor.matmul(
                                pm1,
                                lhsT=w1[:, ci, hc * P:(hc + 1) * P],
                                rhs=h2T[:, ci, tb4 * 256:(tb4 + 1) * 256],
                                start=(ci == 0),
                                stop=(ci == CC - 1),
                            )
                        nc.scalar.activation(
                            gt[:, hc], pm1, AF.Gelu, bias=b1c[:, hc:hc + 1],
                            scale=1.0,
                        )
                    for t2 in range(2):
                        t8 = tb4 * 2 + t2
                        x2r = load_x_tile(b * TB + t8, src=x2_s)
                        outt = work.tile([P, C], F32, tag="outt", name="outt",
                                         bufs=1)
                        pool, ptag = (pden, "pden") if t2 == 0 else (psS, "psS")
                        pm2 = [
                            pool.tile([P, 384], F32, tag=ptag, name="pm2")
                            for _ in range(2)
                        ]
                        for hc in range(HC):
                            for ha in range(2):
                                nc.tensor.matmul(
                                    pm2[ha],
                                    lhsT=gt[:, hc, t2 * P:(t2 + 1) * P],
                                    rhs=w2[:, hc, ha * 384:(ha + 1) * 384],
                                    start=(hc == 0),
                                    stop=False,
                                )
                        for ha in range(2):
                            nc.tensor.matmul(
                                pm2[ha],
                                lhsT=ones1,
                                rhs=b2r[:, ha * 384:(ha + 1) * 384],
                                start=False,
                                stop=True,
                            )
                            nc.vector.tensor_tensor(
                                outt[:, ha * 384:(ha + 1) * 384],
                                pm2[ha],
                                x2r[:, ha * 384:(ha + 1) * 384],
                                ALU.add,
                            )
                        for j in range(2):
                            nc.sync.dma_start(
                                out_d[(b * TB + t8) * P:(b * TB + t8 + 1) * P,
                                      j * 384:(j + 1) * 384],
                                outt[:, j * 384:(j + 1) * 384],
                            )

            # ============ batch 0: LN1 + QKV (resident) ============
            hT0 = big.tile([P, CC, SEQ], BF16, tag="hT", name="hT0")
            qT0 = big.tile([P, CC, SEQ], BF16, tag="qT", name="qT0")
            kT0 = big.tile([P, CC, SEQ], BF16, tag="kT", name="kT0")
            vb0 = big.tile([P, TB, H, D + 1], BF16, tag="vb", name="vb0")
            nc.vector.memset(vb0[:, :, :, D:], 1.0)
            for tb in range(2):
                for t8 in range(tb * 4, tb * 4 + 4):
                    ln1_tile(t8, hT0, t8, pre=pre_x.get(t8))
                    pmv = v_matmuls(hT0, t8)
                    for ha in range(2):
                        nc.vector.tensor_copy(
                            vb0[:, t8, ha * 6:(ha + 1) * 6, :D],
                            pmv[ha].rearrange("p (h d) -> p h d", d=D),
                        )
                for wsb, bsb, dst in ((wq, bq, qT0), (wk, bk, kT0)):
                    for co in range(CC):
                        pm = qk_matmul(wsb, hT0, co, tb)
                        nc.vector.tensor_scalar_add(
                            dst[:, co, tb * 512:(tb + 1) * 512], pm,
                            bsb[:, co:co + 1],
                        )

            # ============ attention(0) ============
            ctxT0 = big.tile([P, CC, SEQ], BF16, tag="ctxT", name="ctxT0")
            attention(qT0, kT0, vb0, ctxT0)

            # ============ batch 1: LN1 + QKV -> DRAM staging ============
            hT1 = big.tile([P, CC, SEQ], BF16, tag="hT", name="hT1")
            for tb in range(2):
                for t8 in range(tb * 4, tb * 4 + 4):
                    ln1_tile(TB + t8, hT1, t8)
                    pmv = v_matmuls(hT1, t8)
                    for ha in range(2):
                        vst = work.tile([P, 6, D + 1], BF16, tag="vst",
                                        name="vst")
                        nc.vector.tensor_copy(
                            vst[:, :, :D],
                            pmv[ha].rearrange("p (h d) -> p h d", d=D),
                        )
                        nc.vector.memset(vst[:, :, D:], 1.0)
                        nc.sync.dma_start(
                            v_s[t8 * P:(t8 + 1) * P, ha * 6:(ha + 1) * 6, :],
                            vst,
                        )
                for wsb, bsb, dst in ((wq, bq, qT_s), (wk, bk, kT_s)):
                    for co in range(CC):
                        pm = qk_matmul(wsb, hT1, co, tb)
                        stg = work.tile([P, 512], BF16, tag="qks", name="stg")
                        nc.vector.tensor_scalar_add(stg, pm, bsb[:, co:co + 1])
                        nc.sync.dma_start(
                            dst[co * P:(co + 1) * P, tb * 512:(tb + 1) * 512],
                            stg,
                        )

            # late weight loads
            wo = wts.tile([P, CC, C], BF16, tag="wo", name="wo")
            load_split(wo, wo_d, CC)
            w1 = wts.tile([P, CC, HID], BF16, tag="w1", name="w1")
            for c in range(CC):
                for hh in range(2):
                    nc.sync.dma_start(
                        w1[:, c, hh * 1536:(hh + 1) * 1536],
                        w1_d[:, c, hh * 1536:(hh + 1) * 1536],
                    )
            w2 = wts.tile([P, HC, C], BF16, tag="w2", name="w2")
            for c in range(HC // 2):
                nc.sync.dma_start(
                    w2[:, c * 2:(c + 1) * 2, :], w2_d[:, c * 2:(c + 1) * 2, :]
                )

            # ============ proj/LN2 (0) ============
            h2T0 = big.tile([P, CC, SEQ], BF16, tag="hT", name="h2T0")
            proj_ln2(ctxT0, wo, 0, h2T0)

            # ============ reload qkv(1) ============
            qT1 = big.tile([P, CC, SEQ], BF16, tag="qT", name="qT1")
            for c in range(CC):
                nc.sync.dma_start(
                    qT1[:, c], qT_s[c * P:(c + 1) * P, :]
                )
            kT1 = big.tile([P, CC, SEQ], BF16, tag="kT", name="kT1")
            for c in range(CC):
                nc.sync.dma_start(
                    kT1[:, c], kT_s[c * P:(c + 1) * P, :]
                )
            vb1 = big.tile([P, TB, H, D + 1], BF16, tag="vb", name="vb1")
            for hh in range(2):
                nc.sync.dma_start(
                    vb1[:, hh * 4:(hh + 1) * 4],
                    v_s[:, :, :].rearrange("(t p) h e -> p t h e", p=P)[
                        :, hh * 4:(hh + 1) * 4
                    ],
                )

            # ============ attention(1) then MLP(0): scheduler interleaves ====
            ctxT1 = big.tile([P, CC, SEQ], BF16, tag="ctxT", name="ctxT1")
            attention(qT1, kT1, vb1, ctxT1)
            mlp(h2T0, w1, w2, 0)

            # ============ proj/LN2 (1) + MLP(1) ============
            h2T1 = big.tile([P, CC, SEQ], BF16, tag="hT", name="h2T1")
            proj_ln2(ctxT1, wo, 1, h2T1)
            mlp(h2T1, w1, w2, 1)

    nc.compile()
    return nc


_NC_CACHE = {}


def _get_nc():
    if "nc" not in _NC_CACHE:
        _NC_CACHE["nc"] = build_nc()
    return _NC_CACHE["nc"]


def _prep_weights(inputs):
    bf = ml_dtypes.bfloat16
    f32 = np.float32
    g1 = inputs["ln1_g"].astype(f32)
    b1_ = inputs["ln1_b"].astype(f32)
    g2 = inputs["ln2_g"].astype(f32)
    b2_ = inputs["ln2_b"].astype(f32)

    def fold(W, bias, g, b, scale=1.0):
        Wf = (g[:, None] * W.astype(f32)) * scale
        bf_ = (b @ W.astype(f32) + bias.astype(f32)) * scale
        return Wf, bf_

    Wq, bq = fold(inputs["Wq"], inputs["bq"], g1, b1_, SCALE)
    Wk, bk = fold(inputs["Wk"], inputs["bk"], g1, b1_)
    Wv, bv = fold(inputs["Wv"], inputs["bv"], g1, b1_)
    W1, b1b = fold(inputs["W1"], inputs["b1"], g2, b2_)
    Wo = inputs["Wo"].astype(f32)
    bo = inputs["bo"].astype(f32)
    W2 = inputs["W2"].astype(f32)
    b2b = inputs["b2"].astype(f32)

    def wchunk(W):  # [C_in, N] -> [128, C_in//128, N]
        return np.ascontiguousarray(
            W.reshape(-1, P, W.shape[1]).transpose(1, 0, 2)
        ).astype(bf)

    def bcol(bias):  # [N] -> [128, N//128]
        return np.ascontiguousarray(bias.reshape(-1, P).T).astype(f32)

    return {
        "wq": wchunk(Wq), "wk": wchunk(Wk), "wv": wchunk(Wv),
        "wo": wchunk(Wo), "w1": wchunk(W1), "w2": wchunk(W2),
        "bq": bcol(bq), "bk": bcol(bk), "b1": bcol(b1b),
        "bv": bv.reshape(1, C).astype(bf),
        "bo": bo.reshape(1, C).astype(bf),
        "b2": b2b.reshape(1, C).astype(bf),
        "ones1": np.ones((1, P), dtype=bf),
        "ident": np.eye(P, dtype=bf),
    }


def kernel(trace=False, **inputs):
    nc = _get_nc()
    shared = _prep_weights(inputs)
    x = np.asarray(inputs["x"], dtype=np.float32)
    B, N, _ = x.shape
    per_core = B // N_CORES
    in_maps = []
    for i in range(N_CORES):
        m = dict(shared)
        m["x"] = np.ascontiguousarray(
            x[i * per_core:(i + 1) * per_core].reshape(per_core * N, C)
        )
        in_maps.append(m)
    res = run_bass_kernel_spmd(
        nc, in_maps, core_ids=list(range(N_CORES)), trace=trace
    )
    out = np.concatenate(
        [r["out"].reshape(per_core, N, C) for r in res.results], axis=0
    )
    if trace:
        kernel.last_results = res
    return out
